# revision 12
# baseline (speedup 1.0000x reference)
"""Trainium2 Bass kernel for single-head attention (no V projection).

Reference computation (per batch b):
    q = x @ Wq ; k = x @ Wk
    scores = q @ k.T / sqrt(64)
    out = softmax(scores, axis=-1) @ x

Shapes: x [4, 2048, 1024], Wq/Wk [1024, 1024] -> out [4, 2048, 1024] fp32.

Key algebraic reduction: with no V projection, scores = x (Wq Wk^T / 8)
x^T, so the host precomputes M = Wq @ Wk.T / 8 (one 1024^3 sgemm in
numpy) and the device never projects k at all.  Each core projects only
its own 1024 query rows (y = x_q @ M) instead of q plus the full
2048-row k.  Per-core PE work: y-proj + scores + attn@x = 640 matmuls
of [128x128]@[128x512], which stream back-to-back at ~227ns each
(gapless in steady state; measured).

Sharding: 8 cores, core c handles batch b=c//2, query-row half h=c%2.
Each core receives its batch's x rolled so its 1024 query rows come
first (attention is permutation-invariant over keys).  No collectives.

v2 changes vs the fp32r baseline (179.8us):
 - All matmul operands are bf16 (same PE rate as fp32r - 1 cycle/row -
   but half the DMA bytes and SBUF).  Simulated end-to-end rel err
   8.1e-3 vs the 2e-2 budget; fp8 DoubleRow was simulated at 3.6e-2
   and is numerically infeasible.
 - Input DMA is coalesced into 7 large units with host-permuted DRAM
   layouts so every unit has >=2KB DRAM-side contiguity, because the
   head was ISSUE-bound: DIRECT2D descriptors issue serially at
   0.6-0.9us each on a queue, and the old schedule needed 9 units
   before the first real matmul.  Units alternate between the two
   HWDGE queues (sync + scalar; scalar's sequencer reaches "main"
   ~0.9us before sync's and so carries the first unit).
 - Output is bf16 (host upcasts), written per 128-row chunk as one
   [128,1024] DMA (2KB rows, full rate) on the scalar queue; the final
   chunk is split into contiguous 64KB pieces landing in a separate
   tile-major param (outf) via both queues in parallel to shrink the
   post-last-matmul tail.

On-chip dataflow (all matmuls contract over the partition dim):
    yT  [d, s]  = M.T @ x_q.T  (lhsT=wt tile, rhs=xT)
    scoresT [t, s] = xT.T-chunks @ yT   (lhsT=xT, rhs=yT)
    expT = Exp(scoresT)        (ScalarE eviction from PSUM, bf16)
    sumexp [s, 2] = partial.T @ ones    (partial = DVE chain of adds)
    out [s, d] = expT.T @ x            (lhsT=expT, rhs=xc)
    out scaled by 1/sumexp on the DVE during PSUM->SBUF eviction.

A burst of warmup matmuls on uninitialized SBUF (no DMA dependency)
runs from ~6.8us so the PE's HAM clock-gate reaches full speed and the
PE never idles >3.4us (which would re-gate it) before the first real
matmuls issue when their inputs land.  The DVFS bring-up (~10us of
reduced-rate matmuls after power onset) is conserved no matter how the
work onset is arranged (measured in many variants).

Softmax skips the max-subtraction: scores have std ~4 and |max| < ~25,
so exp stays comfortably inside fp32/bf16 range and the result is
mathematically identical to jax.nn.softmax.
"""

from contextlib import ExitStack

import numpy as np
import ml_dtypes

import concourse.bacc as bacc
import concourse.tile as tile
from concourse import mybir
from concourse.bass_utils import run_bass_kernel_spmd

F32 = mybir.dt.float32
F32R = mybir.dt.float32r
BF16 = mybir.dt.bfloat16
AFT = mybir.ActivationFunctionType
NP_BF16 = ml_dtypes.bfloat16

P = 128      # partitions
S = 2048     # keys (t) per batch
SQ = 1024    # query rows per core
D = 1024     # model dim
NT = S // P  # 16 t-chunks
ND = D // P  # 8 d-chunks
SB = 512     # query-block width in phase C
NSB = SQ // SB
# Warmup matmuls on uninitialized SBUF - no DMA dependency, so they
# start at ~6.8us (engine boot) and release the PE HAM clock-gate,
# sized to end as the first y-proj inputs land (~9us with fine-grained
# dual-queue DMA issue).
N_WARMUP_A = 6

B_FULL, S_FULL, D_FULL = 4, 2048, 1024
N_CORES = 8

_NC_CACHE = None
LAST_RESULT = None  # BassKernelResults of the most recent kernel() call
TRACE = False      # set by test.py to capture an NTFF profile
TRACE_DIR = None


def _r(ap):
    return ap.bitcast(F32R)


def _build_nc():
    global _NC_CACHE
    if _NC_CACHE is not None:
        return _NC_CACHE

    nc = bacc.Bacc("TRN2")
    # Host-permuted input layouts; every DMA unit below reads >=2KB
    # contiguous DRAM per partition row.
    # xtq[p, sh, dd, c] = x^T[dd*128+p, sh*512+c]      (query cols of x^T)
    xtq = nc.declare_dram_parameter("xtq", [P, 2, ND, 512], BF16, isOutput=False)
    # xtk[p, dd, c] = x^T[dd*128+p, 1024+c]            (key-only cols)
    xtk = nc.declare_dram_parameter("xtk", [P, ND, SQ], BF16, isOutput=False)
    # xp[p, t, c] = x[t*128+p, c]                      (natural, t-chunked)
    xp = nc.declare_dram_parameter("xp", [P, NT, D], BF16, isOutput=False)
    # mg[p, e, dd*128+j] = m[dd*128+p, e*128+j]        (e-major M blocks)
    mg = nc.declare_dram_parameter("mg", [P, ND, D], BF16, isOutput=False)
    ones = nc.declare_dram_parameter("ones", [P, 2], BF16, isOutput=False)
    out = nc.declare_dram_parameter("out", [SQ, D], BF16, isOutput=True)
    # Final output chunk (rows 896:1024, cols 512:1024) lands in its own
    # contiguous 128KB region: one eviction + one DMA after the last
    # matmul (two engines evicting the same PSUM bank serialize on the
    # bank's single read port, so splitting the final eviction loses).
    outf = nc.declare_dram_parameter("outf", [P, 512], BF16, isOutput=True)

    with tile.TileContext(nc) as tc, ExitStack() as ctx:
        singles = ctx.enter_context(tc.tile_pool(name="singles", bufs=1))
        ot = singles.tile([P, 2], BF16)

        persist = ctx.enter_context(tc.tile_pool(name="persist", bufs=1))
        # yT3[:, e, s]: y^T d-chunk e, free axis = query s
        yT = persist.tile([P, ND, SQ], BF16, tag="yT")
        # xT3[:, e, s]: x^T d-chunk e; query rows first (s<1024), key-only
        # rows after.  Persists into phase C as the scores lhsT.
        xT = persist.tile([P, ND, S], BF16, tag="xT")
        # Shared matmul-accumulator pool for the y projection AND the
        # scores groups: keeping one rotation means the first scores
        # group lands in the least-recently-evicted bank instead of
        # waiting on the final y-proj eviction.
        ps_main = ctx.enter_context(
            tc.tile_pool(name="ps_main", bufs=4, space="PSUM"))

        # x natural: t-chunk t at [:, t, :].  Allocated up here because its
        # fill DMAs are issued at the end of phase B's input stream.
        xc_pool = ctx.enter_context(tc.tile_pool(name="xc", bufs=1))
        xc = xc_pool.tile([P, NT, D], BF16)

        # ---------- phase B: load xt/m, project y ----------
        with tc.tile_pool(name="wA", bufs=1) as wA_pool, \
             tc.tile_pool(name="ps_w", bufs=1, space="PSUM") as ps_w_pool:
            wt = wA_pool.tile([P, ND, D], BF16, tag="m")

            # Warmup A: clock-ramp matmuls on uninitialized yT contents
            # (yT's first real write is a DVE eviction much later, so no
            # dependency and no delay); garbage results land in a scratch
            # PSUM bank that is never read.
            ps_w = ps_w_pool.tile([P, 512], F32)
            for i in range(N_WARMUP_A):
                nc.tensor.matmul(ps_w[:], yT[:, 0, 0:P], yT[:, 1, 0:512],
                                 start=(i == 0), stop=(i == N_WARMUP_A - 1))

            # Input DMA: fine-grained units issued in exact NEED order.
            # Units transfer concurrently across the 16 DMA queues and
            # fair-share HBM bandwidth, so the serial ~0.65us-per-unit
            # descriptor issue is the de-facto prioritizer: keep units
            # small and ordered by first use, on both HWDGE queues in
            # parallel.  The first y-proj group's deps (mg-e0 + the dd0
            # stripe) are the two queues' first units; stripes then
            # alternate queues so arrival paces the dd-accumulation chain.
            nc.scalar.dma_start(out=wt[:, 0, :], in_=mg[:, 0, :])
            for sh in range(2):
                for dd in range(0, ND, 2):
                    nc.sync.dma_start(
                        out=xT[:, dd, sh * 512:(sh + 1) * 512],
                        in_=xtq[:, sh, dd, :])
                    nc.scalar.dma_start(
                        out=xT[:, dd + 1, sh * 512:(sh + 1) * 512],
                        in_=xtq[:, sh, dd + 1, :])
            for e in range(1, ND, 2):
                nc.sync.dma_start(out=wt[:, e, :], in_=mg[:, e, :])
                if e + 1 < ND:
                    nc.scalar.dma_start(out=wt[:, e + 1, :], in_=mg[:, e + 1, :])
            nc.scalar.dma_start(out=ot[:], in_=ones[:])
            for dd in range(0, ND, 2):
                nc.sync.dma_start(out=xT[:, dd, SQ:S], in_=xtk[:, dd, :])
                nc.scalar.dma_start(out=xT[:, dd + 1, SQ:S], in_=xtk[:, dd + 1, :])
            for tq in range(0, NT, 8):
                nc.sync.dma_start(
                    out=xc[:, tq:tq + 4, :], in_=xp[:, tq:tq + 4, :])
                nc.scalar.dma_start(
                    out=xc[:, tq + 4:tq + 8, :], in_=xp[:, tq + 4:tq + 8, :])

            # yT projection; 512-wide stripes only - narrower ones are
            # LDWEIGHTS-bound.
            for e in range(ND):
                for sh in range(SQ // 512):
                    ps = ps_main.tile([P, 512], F32)
                    for dd in range(ND):
                        nc.tensor.matmul(
                            ps[:],
                            wt[:, e, dd * P:(dd + 1) * P],
                            xT[:, dd, sh * 512: sh * 512 + 512],
                            start=(dd == 0), stop=(dd == ND - 1),
                        )
                    nc.vector.tensor_copy(
                        yT[:, e, sh * 512: sh * 512 + 512], ps[:],
                    )

        # ---------- phase C: scores -> softmax -> attn @ x ----------
        with tc.tile_pool(name="exp", bufs=1) as exp_pool, \
             tc.tile_pool(name="outp", bufs=3) as out_pool, \
             tc.tile_pool(name="outs", bufs=1) as outs_pool, \
             tc.tile_pool(name="recip", bufs=4) as recip_pool, \
             tc.tile_pool(name="partial", bufs=2) as partial_pool, \
             tc.tile_pool(name="ps_av", bufs=3, space="PSUM") as ps_av, \
             tc.tile_pool(name="ps_sum", bufs=1, space="PSUM") as ps_sum:
            for blk in range(NSB):
                # expT3[:, t, s-within-blk]
                expT = exp_pool.tile([P, NT, SB], BF16, tag="expT")
                # Softmax denominator: the 16-chunk accumulation runs on
                # the (otherwise idle) DVE as a chain of adds interleaved
                # with the scores loop; the PE then only does one N=2
                # partition-reduce matmul per s-chunk instead of 16
                # LDWEIGHTS-bound ones.
                partial = partial_pool.tile([P, SB], BF16, tag="partial")
                for t in range(NT):
                    ps = ps_main.tile([P, SB], F32)
                    for e in range(ND):
                        nc.tensor.matmul(
                            ps[:],
                            xT[:, e, t * P: (t + 1) * P],
                            yT[:, e, blk * SB: (blk + 1) * SB],
                            start=(e == 0), stop=(e == ND - 1),
                        )
                    nc.scalar.activation(expT[:, t, :], ps[:], AFT.Exp)
                    if t == 1:
                        nc.vector.tensor_add(
                            partial[:], expT[:, 0, :], expT[:, 1, :])
                    elif t >= 2:
                        nc.vector.tensor_add(
                            partial[:], partial[:], expT[:, t, :])

                for ss in range(SB // P):
                    pss = ps_sum.tile([P, 2], F32)
                    nc.tensor.matmul(
                        pss[:], partial[:, ss * P:(ss + 1) * P], ot[:],
                        start=True, stop=True,
                    )
                    rec = recip_pool.tile([P, 1], F32, tag="rec")
                    nc.vector.reciprocal(rec[:], pss[:, 0:1])

                    last_ss = (blk == NSB - 1 and ss == SB // P - 1)
                    row0 = blk * SB + ss * P
                    # One [P, 1024] output tile per ss-chunk: both dh
                    # halves evict into it (their evictions are naturally
                    # ordered), then a single full-rate 2KB-row DMA.
                    if not last_ss:
                        ob = out_pool.tile([P, 1024], BF16, tag="ob")
                    for dh in range(2):
                        psa = ps_av.tile([P, 512], F32)
                        for t in range(NT):
                            nc.tensor.matmul(
                                psa[:],
                                expT[:, t, ss * P: (ss + 1) * P],
                                xc[:, t, dh * 512: dh * 512 + 512],
                                start=(t == 0), stop=(t == NT - 1),
                            )
                        if last_ss and dh == 0:
                            # Evict + DMA the first half immediately; its
                            # 1KB-row DMA hides under the dh=1 matmuls.
                            obs = outs_pool.tile([P, 512], BF16, tag="obs")
                            nc.vector.tensor_scalar_mul(
                                obs[:], psa[:], rec[:, 0:1])
                            nc.scalar.dma_start(
                                out=out[row0:row0 + P, 0:512], in_=obs[:])
                        elif last_ss:
                            # Final block: one DVE eviction + one DMA into
                            # a contiguous 128KB region on the idle sync
                            # queue - the minimal exposed tail.
                            oba = out_pool.tile([P, 512], BF16, tag="oba")
                            nc.vector.tensor_scalar_mul(
                                oba[:], psa[:], rec[:, 0:1])
                            nc.sync.dma_start(out=outf[:, :], in_=oba[:])
                        else:
                            nc.vector.tensor_scalar_mul(
                                ob[:, dh * 512:dh * 512 + 512], psa[:],
                                rec[:, 0:1])
                            if dh == 1:
                                nc.scalar.dma_start(
                                    out=out[row0:row0 + P, :], in_=ob[:])

    nc.finalize()
    _NC_CACHE = nc
    return nc


def kernel(inputs, Wq, Wk):
    global LAST_RESULT
    x = np.asarray(inputs, dtype=np.float32)
    assert x.shape == (B_FULL, S_FULL, D_FULL)
    # scores = x (Wq Wk^T / 8) x^T: fold the projections and the softmax
    # scale into one host-side sgemm.
    m = (np.asarray(Wq, dtype=np.float32) @ np.asarray(Wk, dtype=np.float32).T) \
        * np.float32(0.125)
    # e-major blocks: mg[p, e, dd*128+j] = m[dd*128+p, e*128+j]
    mgf = m.reshape(ND, P, ND, P).transpose(1, 2, 0, 3).reshape(P, ND, D)
    mg = np.ascontiguousarray(mgf).astype(NP_BF16)
    ones = np.ones((P, 2), dtype=NP_BF16)

    nc = _build_nc()

    in_maps = []
    for c in range(N_CORES):
        b, h = c // 2, c % 2
        xb = x[b]
        if h:
            xb = np.concatenate([xb[SQ:], xb[:SQ]], axis=0)
        xb16 = xb.astype(NP_BF16)
        xt16 = np.ascontiguousarray(xb16.T)  # [D, S]
        # xtq[p, sh, dd, c] = xt16[dd*128+p, sh*512+c]
        xtq = np.ascontiguousarray(
            xt16[:, 0:SQ].reshape(ND, P, 2, 512).transpose(1, 2, 0, 3))
        # xtk[p, dd, c] = xt16[dd*128+p, 1024+c]
        xtk = np.ascontiguousarray(
            xt16[:, SQ:S].reshape(ND, P, SQ).transpose(1, 0, 2))
        # xp[p, t, c] = xb16[t*128+p, c]
        xpa = np.ascontiguousarray(
            xb16.reshape(NT, P, D).transpose(1, 0, 2))
        in_maps.append({
            "xtq": xtq,
            "xtk": xtk,
            "xp": xpa,
            "mg": mg,
            "ones": ones,
        })

    kwargs = {}
    if TRACE:
        kwargs = {"trace": True, "tmpdir": TRACE_DIR}
    res = run_bass_kernel_spmd(nc, in_maps, list(range(N_CORES)), **kwargs)
    LAST_RESULT = res

    full = np.empty((B_FULL, S_FULL, D_FULL), dtype=np.float32)
    for c in range(N_CORES):
        b, h = c // 2, c % 2
        o = res.results[c]["out"].astype(np.float32)
        of = res.results[c]["outf"].astype(np.float32)
        o[SQ - P:, 512:1024] = of
        full[b, h * SQ:(h + 1) * SQ, :] = o
    return full
